# revision 1
# baseline (speedup 1.0000x reference)
"""Trainium2 kernel for nn_Contrast: contrastive loss over a 10000x10000
exp-cosine-similarity matrix, sharded by rows across 8 NeuronCores.

Structure:
  host (tiny, O(N*D)): 8->8->8 MLP projection of both views, row norms,
      fold 1/(n1*n2*tau) into the operands:  a = zp1/n1,  b = zp2/(n2*tau).
      Then m[i,j] = exp(a_i . b_j).
  device (O(N^2)), per core k over its 1280-row slice of a:
      for each [128 x <=512] tile of a_rows @ b^T:
        PE matmul (K=8) -> PSUM (3 tiles share a 3-bank PSUM tensor)
        ACT exp PSUM->SBUF with accum_out => row-sum partials (free)
        PE one-hot matmul (E_c^T @ exp_tile) accumulating column sums for
        all tiles into a single [20, 512] PSUM bank
  host: subtract zero-padding contributions, add eps, diag from exact dots,
      assemble the two mean log-ratio losses.
"""

import numpy as np

import concourse.bass as bass
import concourse.bacc as bacc
import concourse.mybir as mybir
import concourse.tile as tile
from concourse.bass_utils import run_bass_kernel_spmd

TAU = 0.5
LAM = 0.5
EPS = 1e-8

N = 10000
D = 8
NCORES = 8
RPAD = 10240              # lhs rows padded: 8 cores * 1280
RPC = RPAD // NCORES      # rows per core = 1280
NSTRIP = RPC // 128       # 10 strips of 128 rows
ROW_PAD = RPAD - N        # 240 zero lhs rows -> contribute exp(0)=1 per column

# column tiles cover exactly N columns: 19 x 512 + 272
COL_TILES = [(c * 512, min(512, N - c * 512)) for c in range((N + 511) // 512)]
NCT = len(COL_TILES)      # 20
# tiles per strip are grouped so each group's matmuls share one PSUM tensor
# and one ACT(exp) instruction. PSUM budget is 8 banks: 2 x 3-bank tensors
# (double-buffered) + 1 bank for the column-sum accumulator. The short
# (2-tile, 784-wide) group goes first in each strip: a short ACT instruction
# in the middle of a strip stalls the pipeline less there.
GROUPS = [COL_TILES[18:20]] + [COL_TILES[i : i + 3] for i in range(0, 18, 3)]

# "f32" is the exact-but-slow path (PE runs fp32 at 4 cycles/row).
# "f32r" streams fp32 bits through the PE at full rate with relaxed rounding;
# "bf16" is the same speed with coarser rounding and no staging copies.
# All accumulation stays fp32 and the scalar loss averages the per-element
# rounding noise away (measured loss rel err: bf16 0.0, f32r 1e-7, f32 1e-7).
MM_DTYPE = "bf16"


def _mybir_dt(name):
    return {
        "f32": mybir.dt.float32,
        "f32r": mybir.dt.float32r,
        "bf16": mybir.dt.bfloat16,
    }[name]


def _np_dt(name):
    if name in ("f32", "f32r"):
        return np.float32
    import ml_dtypes

    return ml_dtypes.bfloat16


def _build_nc(dt_name):
    dt_in = _mybir_dt(dt_name)
    f32 = mybir.dt.float32
    nc = bacc.Bacc(None)

    dram_dt = mybir.dt.bfloat16 if dt_name == "bf16" else mybir.dt.float32
    lhsT = nc.dram_tensor("lhsT", [D, RPC], dram_dt, kind="ExternalInput")
    rhsT = nc.dram_tensor("rhsT", [D, N], dram_dt, kind="ExternalInput")
    eblk = nc.dram_tensor("eblk", [128, NCT * 20], dram_dt, kind="ExternalInput")
    out_rowsum = nc.dram_tensor("out_rowsum", [128, NSTRIP], f32, kind="ExternalOutput")
    out_colsum = nc.dram_tensor("out_colsum", [20, 512], f32, kind="ExternalOutput")

    ngroups = len(GROUPS)
    n_onehot = NSTRIP * NCT

    with tile.TileContext(nc) as tc:
        with (
            tc.tile_pool(name="inp", bufs=1) as inp_pool,
            tc.tile_pool(name="etile", bufs=4) as etile_pool,
            tc.tile_pool(name="rowp", bufs=2) as rowp_pool,
            tc.tile_pool(name="persist", bufs=1) as persist_pool,
            tc.tile_pool(name="pmm", bufs=2, space="PSUM") as pmm_pool,
            tc.tile_pool(name="pcol", bufs=1, space="PSUM") as pcol_pool,
        ):
            lhsT_sb = inp_pool.tile([D, RPC], dt_in)
            rhsT_sb = inp_pool.tile([D, N], dt_in)
            eblk_sb = inp_pool.tile([128, NCT * 20], dt_in)

            if dt_name == "f32r":
                # f32r operands need a rounding producer; sync-DMA into f32
                # staging, then idle-DVE copies do the cast. Chunked so the
                # first matmuls start as soon as their span is staged; eblk is
                # only needed by the first one-hot matmul (~8us in), so it
                # loads after the first two rhs chunks.
                lhsT_st = inp_pool.tile([D, RPC], f32)
                rhsT_st = inp_pool.tile([D, N], f32)
                eblk_st = inp_pool.tile([128, NCT * 20], f32)

                # each dma_start costs ~650ns of serial sequencer issue, so
                # the pieces feeding the first matmuls go first and the bulk
                # follows in a few large DMAs. DVE cast copies are chunked in
                # group-consumption order so compute starts as data rounds.
                spans = []
                for grp in GROUPS:
                    g0 = grp[0][0]
                    spans.append((g0, g0 + sum(w for _, w in grp)))
                rest = sorted(spans[2:])  # contiguous ascending tail spans
                nc.sync.dma_start(out=lhsT_st[:, 0:128], in_=lhsT[:, 0:128])
                nc.sync.dma_start(
                    out=rhsT_st[:, spans[0][0] : spans[0][1]],
                    in_=rhsT[:, spans[0][0] : spans[0][1]],
                )
                nc.sync.dma_start(
                    out=rhsT_st[:, spans[1][0] : spans[1][1]],
                    in_=rhsT[:, spans[1][0] : spans[1][1]],
                )
                nc.sync.dma_start(out=lhsT_st[:, 128:RPC], in_=lhsT[:, 128:RPC])
                nc.sync.dma_start(
                    out=rhsT_st[:, rest[0][0] : rest[2][1]],
                    in_=rhsT[:, rest[0][0] : rest[2][1]],
                )
                nc.sync.dma_start(out=eblk_st[:], in_=eblk[:])
                nc.sync.dma_start(
                    out=rhsT_st[:, rest[3][0] : rest[-1][1]],
                    in_=rhsT[:, rest[3][0] : rest[-1][1]],
                )

                def _cast(dst, st, lo, hi):
                    nc.vector.tensor_copy(out=dst[:, lo:hi], in_=st[:, lo:hi])

                _cast(lhsT_sb, lhsT_st, 0, 128)
                _cast(rhsT_sb, rhsT_st, *spans[0])
                _cast(lhsT_sb, lhsT_st, 128, RPC)
                _cast(rhsT_sb, rhsT_st, *spans[1])
                _cast(eblk_sb, eblk_st, 0, NCT * 20)
                for sp in spans[2:]:
                    _cast(rhsT_sb, rhsT_st, *sp)
            else:
                nc.sync.dma_start(out=lhsT_sb[:], in_=lhsT[:])
                for grp in GROUPS[:2]:
                    g0 = grp[0][0]
                    gw = sum(w for _, w in grp)
                    nc.sync.dma_start(
                        out=rhsT_sb[:, g0 : g0 + gw], in_=rhsT[:, g0 : g0 + gw]
                    )
                nc.sync.dma_start(out=eblk_sb[:], in_=eblk[:])
                for grp in GROUPS[2:]:
                    g0 = grp[0][0]
                    gw = sum(w for _, w in grp)
                    nc.sync.dma_start(
                        out=rhsT_sb[:, g0 : g0 + gw], in_=rhsT[:, g0 : g0 + gw]
                    )

            rowsum_sb = persist_pool.tile([128, NSTRIP], f32)
            colsum_sb = persist_pool.tile([20, 512], f32)
            colp = pcol_pool.tile([20, 512], f32)

            # software-pipeline the one-hot (column-sum) matmuls two groups
            # behind the main matmuls: at strip boundaries PE then runs the
            # next strip's main matmuls before the deferred one-hots, so ACT
            # is never left waiting on PE's in-order queue
            pending = []
            onehot_idx = 0

            def flush_one(et, grp):
                nonlocal onehot_idx
                off = 0
                for c0, w in grp:
                    c = c0 // 512  # global column-tile index = colp row
                    nc.tensor.matmul(
                        colp[:, 0:w],
                        eblk_sb[:, c * 20 : (c + 1) * 20],
                        et[:, off : off + w],
                        start=(onehot_idx == 0),
                        stop=(onehot_idx == n_onehot - 1),
                        skip_group_check=True,
                    )
                    off += w
                    onehot_idx += 1

            def flush_pending(keep=0):
                while len(pending) > keep:
                    flush_one(*pending.pop(0))

            for r in range(NSTRIP):
                rowp = rowp_pool.tile([128, ngroups], f32)
                for gi, grp in enumerate(GROUPS):
                    gw = sum(w for _, w in grp)
                    pa = pmm_pool.tile([128, 1536], f32, name=f"pa_{r}_{gi}", tag="pa")
                    off = 0
                    for c0, w in grp:
                        nc.tensor.matmul(
                            pa[:, off : off + w],
                            lhsT_sb[:, r * 128 : (r + 1) * 128],
                            rhsT_sb[:, c0 : c0 + w],
                            start=True,
                            stop=True,
                        )
                        off += w
                    et = etile_pool.tile([128, 1536], dt_in)
                    nc.scalar.activation(
                        et[:, :gw],
                        pa[:, :gw],
                        mybir.ActivationFunctionType.Exp,
                        accum_out=rowp[:, gi : gi + 1],
                    )
                    flush_pending(keep=1)
                    pending.append((et, grp))
                nc.vector.reduce_sum(
                    out=rowsum_sb[:, r : r + 1],
                    in_=rowp[:, :],
                    axis=mybir.AxisListType.X,
                )
            flush_pending()

            nc.vector.tensor_copy(out=colsum_sb[:], in_=colp[:])
            nc.sync.dma_start(out=out_rowsum[:], in_=rowsum_sb[:])
            nc.sync.dma_start(out=out_colsum[:], in_=colsum_sb[:])

    nc.compile()
    return nc


_NC_CACHE = {}


def _get_nc(dt_name):
    if dt_name not in _NC_CACHE:
        _NC_CACHE[dt_name] = _build_nc(dt_name)
    return _NC_CACHE[dt_name]


def _proj_np(z, W1, b1, W2, b2):
    h = z @ W1.T + b1
    h = np.where(h > 0, h, np.expm1(h)).astype(np.float32)
    return (h @ W2.T + b2).astype(np.float32)


def _prepare_operands(z_mp, z_sc, W1, b1, W2, b2):
    zp1 = _proj_np(z_mp.astype(np.float32), W1, b1, W2, b2)
    zp2 = _proj_np(z_sc.astype(np.float32), W1, b1, W2, b2)
    n1 = np.sqrt(np.sum(zp1 * zp1, axis=1, keepdims=True)).astype(np.float32)
    n2 = np.sqrt(np.sum(zp2 * zp2, axis=1, keepdims=True)).astype(np.float32)
    a = (zp1 / n1).astype(np.float32)
    b = (zp2 / (n2 * np.float32(TAU))).astype(np.float32)
    dots = np.sum(a * b, axis=1).astype(np.float32)  # diag logits (exact path)
    return a, b, dots


def _make_in_maps(a, b):
    np_dt = _np_dt(MM_DTYPE)
    a_pad = np.zeros((RPAD, D), np.float32)
    a_pad[:N] = a
    aT = np.ascontiguousarray(a_pad.T).astype(np_dt)
    bT = np.ascontiguousarray(b.T).astype(np_dt)
    E = np.ascontiguousarray(
        np.tile(np.eye(20, dtype=np_dt)[None], (128, 1, 1)).reshape(128, NCT * 20)
    )
    return [
        {
            "lhsT": np.ascontiguousarray(aT[:, k * RPC : (k + 1) * RPC]),
            "rhsT": bT,
            "eblk": E,
        }
        for k in range(NCORES)
    ]


def _finalize(res, dots):
    rowsum_full = np.concatenate(
        [np.asarray(res[k]["out_rowsum"]).T.reshape(-1) for k in range(NCORES)]
    )
    colsum_full = np.sum(
        [np.asarray(res[k]["out_colsum"]).reshape(-1) for k in range(NCORES)], axis=0
    )
    row_sum = rowsum_full[:N].astype(np.float64) + EPS
    col_sum = colsum_full[:N].astype(np.float64) - ROW_PAD + EPS
    diag = np.exp(dots.astype(np.float64))
    lori_mp = -np.mean(np.log(diag / row_sum))
    lori_sc = -np.mean(np.log(diag / col_sum))
    return np.float32(LAM * lori_mp + (1.0 - LAM) * lori_sc)


def kernel(z_mp, z_sc, W1, b1, W2, b2):
    a, b, dots = _prepare_operands(z_mp, z_sc, W1, b1, W2, b2)
    in_maps = _make_in_maps(a, b)
    nc = _get_nc(MM_DTYPE)
    res = run_bass_kernel_spmd(nc, in_maps, list(range(NCORES))).results
    return _finalize(res, dots)



# revision 2
# speedup vs baseline: 3.2614x; 3.2614x over previous
"""Trainium2 kernel for nn_Contrast: contrastive loss over the 10000x10000
exp-cosine-similarity matrix, via a polynomial kernel-feature expansion.

The loss only consumes the similarity matrix through per-row and per-column
sums of m = exp(a.b^T) (a = zp1/n1, b = zp2/(n2*tau)), plus the exact
diagonal.  exp is replaced by a least-squares polynomial p(x) = sum c_k x^k
fit on the empirical similarity distribution (deg 3 -> loss rel err ~4e-5,
tolerance 2e-2).  With phi the vector of monomials of degree 1..DEG in the
8 coordinates,

    rowsum_i ~= c0*N + sum_alpha w_alpha phi_alpha(a_i) * Psi_alpha,
    Psi_alpha = sum_j phi_alpha(b_j),   w_alpha = c_|alpha| * multinomial(alpha)

and symmetrically colsum_j with Phi = sum_i phi_alpha(a_i).  This is O(N*NF)
instead of O(N^2): no N x N matrix and no 1e8 exp() evaluations.

Device structure (rows sharded 1250/core across 8 cores, two launches):
  P1: DVE generates phi for the core's a- and b-shard (monomial recursion,
      strip-batched tensor_tensor with a broadcast coordinate operand);
      PE reduces over rows (ones-matmul, PSUM-accumulated over 10 strips)
      -> per-core partials [Phi | Psi].  Host sums the 8 tiny partials.
  P2: regenerates phi, then DVE scalar_tensor_tensor (all-bf16 packed SBUF
      operands -> 4x DVE mode) multiplies by broadcast w*Psi / w*Phi and
      accum_out emits the row/col sums directly.
Host does only O(N*D) prep (projection, norms, exact diagonal — same as the
exact-kernel baseline), the tiny poly fit, and the O(N) log/mean finalize.
"""

import numpy as np
import ml_dtypes

import concourse.bass as bass
import concourse.bacc as bacc
import concourse.mybir as mybir
import concourse.tile as tile
from concourse.bass_utils import run_bass_kernel_spmd

TAU = 0.5
LAM = 0.5
EPS = 1e-8

N = 10000
D = 8
NCORES = 8
RPC = N // NCORES          # 1250 real rows per core
NSTRIP = 10                # 10 strips x 128 partitions = 1280 slots (30 pad)
SLOTS = NSTRIP * 128
DEG = 3

BF16 = ml_dtypes.bfloat16


def _build_recipe():
    """Monomial ordering: degree-major; within a degree, grouped by max
    variable index so each degree-k/maxvar-d block is (prefix of the
    degree-(k-1) block) * x_d.  Returns (mons, ops) where ops entries are
    (in_off, out_off, g, d) with offsets into the full monomial list."""
    mons = [(d,) for d in range(D)]
    ops = []
    prev_start, prev_len = 0, D
    for k in range(2, DEG + 1):
        out_start = len(mons)
        for d in range(D):
            g = sum(1 for m in mons[prev_start:prev_start + prev_len] if max(m) <= d)
            if g == 0:
                continue
            ops.append((prev_start, len(mons), g, d))
            for m in mons[prev_start:prev_start + g]:
                mons.append(tuple(sorted(m + (d,))))
        prev_start, prev_len = out_start, len(mons) - out_start
    return mons, ops


MONS, GEN_OPS = _build_recipe()
NF = len(MONS)             # 164 for DEG=3


def _multinom(m):
    from math import factorial
    counts = {}
    for v in m:
        counts[v] = counts.get(v, 0) + 1
    r = factorial(len(m))
    for c in counts.values():
        r //= factorial(c)
    return r


MULTINOM = np.array([_multinom(m) for m in MONS], np.float64)
MON_DEG = np.array([len(m) for m in MONS], np.int64)


def _emit_gen(nc, F3):
    """Emit the monomial-generation ops for both views into F3
    [128, NSTRIP, 2*NF] (view A at free offset 0, view B at NF)."""
    for v in range(2):
        base = v * NF
        for (in_off, out_off, g, d) in GEN_OPS:
            nc.vector.tensor_tensor(
                out=F3[:, :, base + out_off : base + out_off + g],
                in0=F3[:, :, base + in_off : base + in_off + g],
                in1=F3[:, :, base + d : base + d + 1].broadcast_to([128, NSTRIP, g]),
                op=mybir.AluOpType.mult,
            )


def _build_p1():
    f32 = mybir.dt.float32
    bf16 = mybir.dt.bfloat16
    nc = bacc.Bacc(None)
    coords = nc.dram_tensor("coords", [128, NSTRIP * 2 * D], bf16, kind="ExternalInput")
    out_psi = nc.dram_tensor("psi", [1, 2 * NF], f32, kind="ExternalOutput")

    with tile.TileContext(nc) as tc:
        with (
            tc.tile_pool(name="feat", bufs=1) as feat_pool,
            tc.tile_pool(name="small", bufs=1) as small_pool,
            tc.tile_pool(name="psum", bufs=1, space="PSUM") as psum_pool,
        ):
            F3 = feat_pool.tile([128, NSTRIP, 2 * NF], bf16)
            ones = small_pool.tile([128, 1], bf16)
            psi_sb = small_pool.tile([1, 2 * NF], f32)
            acc = psum_pool.tile([1, 2 * NF], f32)

            nc.vector.memset(ones[:], 1.0)
            # coords dram [p, (s, v, d)] -> degree-1 slots of F3
            nc.sync.dma_start(
                out=F3.rearrange("p s (v x) -> p s v x", v=2)[:, :, :, 0:D],
                in_=coords.rearrange("p (s v d) -> p s v d", v=2, d=D),
            )
            _emit_gen(nc, F3)
            for s in range(NSTRIP):
                nc.tensor.matmul(
                    acc[:, :],
                    ones[:],
                    F3[:, s],
                    start=(s == 0),
                    stop=(s == NSTRIP - 1),
                )
            nc.scalar.copy(out=psi_sb[:], in_=acc[:])
            nc.sync.dma_start(out=out_psi[:], in_=psi_sb[:])

    nc.compile()
    return nc


def _build_p2():
    f32 = mybir.dt.float32
    bf16 = mybir.dt.bfloat16
    nc = bacc.Bacc(None)
    coords = nc.dram_tensor("coords", [128, NSTRIP * 2 * D], bf16, kind="ExternalInput")
    wpair = nc.dram_tensor("wpair", [128, 2 * NF], bf16, kind="ExternalInput")
    out_sums = nc.dram_tensor("sums", [128, 2 * NSTRIP], f32, kind="ExternalOutput")

    with tile.TileContext(nc) as tc:
        with (
            tc.tile_pool(name="feat", bufs=1) as feat_pool,
            tc.tile_pool(name="small", bufs=1) as small_pool,
        ):
            F3 = feat_pool.tile([128, NSTRIP, 2 * NF], bf16)
            w_sb = small_pool.tile([128, 2 * NF], bf16)
            junk = small_pool.tile([128, NF], bf16)
            sums = small_pool.tile([128, 2 * NSTRIP], f32)

            nc.sync.dma_start(
                out=F3.rearrange("p s (v x) -> p s v x", v=2)[:, :, :, 0:D],
                in_=coords.rearrange("p (s v d) -> p s v d", v=2, d=D),
            )
            nc.sync.dma_start(out=w_sb[:], in_=wpair[:])
            _emit_gen(nc, F3)
            for s in range(NSTRIP):
                for v in range(2):
                    nc.vector.scalar_tensor_tensor(
                        out=junk[:],
                        in0=F3[:, s, v * NF : (v + 1) * NF],
                        scalar=1.0,
                        in1=w_sb[:, v * NF : (v + 1) * NF],
                        op0=mybir.AluOpType.mult,
                        op1=mybir.AluOpType.mult,
                        accum_out=sums[:, 2 * s + v : 2 * s + v + 1],
                    )
            nc.sync.dma_start(out=out_sums[:], in_=sums[:])

    nc.compile()
    return nc


_NC_CACHE = {}


def _get_nc(which):
    if which not in _NC_CACHE:
        _NC_CACHE[which] = _build_p1() if which == "p1" else _build_p2()
    return _NC_CACHE[which]


def _proj_np(z, W1, b1, W2, b2):
    h = z @ W1.T + b1
    h = np.where(h > 0, h, np.expm1(h)).astype(np.float32)
    return (h @ W2.T + b2).astype(np.float32)


def _prepare_operands(z_mp, z_sc, W1, b1, W2, b2):
    zp1 = _proj_np(z_mp.astype(np.float32), W1, b1, W2, b2)
    zp2 = _proj_np(z_sc.astype(np.float32), W1, b1, W2, b2)
    n1 = np.sqrt(np.sum(zp1 * zp1, axis=1, keepdims=True)).astype(np.float32)
    n2 = np.sqrt(np.sum(zp2 * zp2, axis=1, keepdims=True)).astype(np.float32)
    a = (zp1 / n1).astype(np.float32)
    b = (zp2 / (n2 * np.float32(TAU))).astype(np.float32)
    dots = np.sum(a.astype(np.float64) * b.astype(np.float64), axis=1)  # exact diag logits
    return a, b, dots


def _fit_poly(a, b):
    """Least-squares fit of exp on a subsample of the actual similarity
    distribution (the only consumer is log(sum), so ~1e-4 sum error is
    orders of magnitude inside the tolerance)."""
    xs = (a[::11].astype(np.float64) @ b[::13].astype(np.float64).T).ravel()
    V = np.vander(xs, DEG + 1, increasing=True)
    G = V.T @ V
    r = V.T @ np.exp(xs)
    return np.linalg.solve(G, r)  # c[0..DEG]


def _make_coords(a, b):
    """Pack per-core coords [128, (s, v, d)] in bf16, zero-padding the 30
    slots beyond the 1250 real rows (monomials of 0 are 0, so pads drop out
    of Psi/Phi automatically)."""
    out = []
    for k in range(NCORES):
        c = np.zeros((SLOTS, 2, D), np.float32)
        c[:RPC, 0, :] = a[k * RPC : (k + 1) * RPC]
        c[:RPC, 1, :] = b[k * RPC : (k + 1) * RPC]
        c = c.reshape(NSTRIP, 128, 2 * D).transpose(1, 0, 2).reshape(128, NSTRIP * 2 * D)
        out.append(np.ascontiguousarray(c.astype(BF16)))
    return out


def kernel(z_mp, z_sc, W1, b1, W2, b2):
    a, b, dots = _prepare_operands(z_mp, z_sc, W1, b1, W2, b2)
    c = _fit_poly(a, b)
    coords = _make_coords(a, b)

    nc1 = _get_nc("p1")
    res1 = run_bass_kernel_spmd(
        nc1, [{"coords": coords[k]} for k in range(NCORES)], list(range(NCORES))
    ).results
    partials = np.sum([np.asarray(res1[k]["psi"])[0] for k in range(NCORES)], axis=0)
    Phi = partials[:NF].astype(np.float64)   # sum_i phi(a_i)
    Psi = partials[NF:].astype(np.float64)   # sum_j phi(b_j)

    w = c[MON_DEG] * MULTINOM
    wpsi = (w * Psi).astype(np.float32)      # weights for the a-side dot (rowsum)
    wphi = (w * Phi).astype(np.float32)      # weights for the b-side dot (colsum)
    wpair = np.ascontiguousarray(
        np.tile(np.concatenate([wpsi, wphi]).astype(BF16)[None, :], (128, 1))
    )

    nc2 = _get_nc("p2")
    res2 = run_bass_kernel_spmd(
        nc2,
        [{"coords": coords[k], "wpair": wpair} for k in range(NCORES)],
        list(range(NCORES)),
    ).results

    row_sum = np.empty(N, np.float64)
    col_sum = np.empty(N, np.float64)
    for k in range(NCORES):
        s = np.asarray(res2[k]["sums"]).astype(np.float64)  # [128, 2*NSTRIP]
        rs = s[:, 0::2].T.reshape(-1)[:RPC]  # [p, s] -> row (s*128+p)
        cs = s[:, 1::2].T.reshape(-1)[:RPC]
        row_sum[k * RPC : (k + 1) * RPC] = rs
        col_sum[k * RPC : (k + 1) * RPC] = cs
    row_sum += c[0] * N + EPS
    col_sum += c[0] * N + EPS

    diag = np.exp(dots)
    lori_mp = -np.mean(np.log(diag / row_sum))
    lori_sc = -np.mean(np.log(diag / col_sum))
    return np.float32(LAM * lori_mp + (1.0 - LAM) * lori_sc)


# revision 6
# speedup vs baseline: 4.0179x; 1.2320x over previous
"""Trainium2 kernel for nn_Contrast: contrastive loss over the 10000x10000
exp-cosine-similarity matrix, via a polynomial kernel-feature expansion.

The loss only consumes the similarity matrix through per-row and per-column
sums of m = exp(a.b^T) (a = zp1/n1, b = zp2/(n2*tau)), plus the exact
diagonal.  exp is replaced by a least-squares polynomial p(x) = sum c_k x^k
fit on the empirical similarity distribution (deg 3 -> loss rel err ~4e-5,
tolerance 2e-2).  With phi the vector of monomials of degree 1..DEG in the
8 coordinates,

    rowsum_i ~= c0*N + sum_alpha w_alpha phi_alpha(a_i) * Psi_alpha,
    Psi_alpha = sum_j phi_alpha(b_j),   w_alpha = c_|alpha| * multinomial(alpha)

and symmetrically colsum_j with Phi = sum_i phi_alpha(a_i).  This is O(N*NF)
instead of O(N^2): no N x N matrix and no 1e8 exp() evaluations.

Device structure (rows sharded 1250/core across 8 cores, two launches):
  P1: DVE generates phi for the core's a- and b-shard (monomial recursion,
      strip-batched tensor_tensor with a broadcast coordinate operand);
      PE reduces over rows (ones-matmul, PSUM-accumulated over 10 strips)
      -> per-core partials [Phi | Psi].  Host sums the 8 tiny partials.
  P2: regenerates phi, then DVE scalar_tensor_tensor (all-bf16 packed SBUF
      operands -> 4x DVE mode) multiplies by broadcast w*Psi / w*Phi and
      accum_out emits the row/col sums directly.
Host does only O(N*D) prep (projection, norms, exact diagonal — same as the
exact-kernel baseline), the tiny poly fit, and the O(N) log/mean finalize.
"""

import numpy as np
import ml_dtypes

import concourse.bass as bass
import concourse.bacc as bacc
import concourse.mybir as mybir
import concourse.tile as tile
from concourse.bass_utils import run_bass_kernel_spmd

TAU = 0.5
LAM = 0.5
EPS = 1e-8

N = 10000
D = 8
NCORES = 8
RPC = N // NCORES          # 1250 real rows per core
NSTRIP = 10                # 10 strips x 128 partitions = 1280 slots (30 pad)
SLOTS = NSTRIP * 128
DEG = 3

BF16 = ml_dtypes.bfloat16


def _build_recipe():
    """Monomial ordering: degree-major; within a degree, grouped by max
    variable index so each degree-k/maxvar-d block is (prefix of the
    degree-(k-1) block) * x_d.  Returns (mons, ops) where ops entries are
    (in_off, out_off, g, d) with offsets into the full monomial list."""
    mons = [(d,) for d in range(D)]
    ops = []
    prev_start, prev_len = 0, D
    for k in range(2, DEG + 1):
        out_start = len(mons)
        for d in range(D):
            g = sum(1 for m in mons[prev_start:prev_start + prev_len] if max(m) <= d)
            if g == 0:
                continue
            ops.append((prev_start, len(mons), g, d))
            for m in mons[prev_start:prev_start + g]:
                mons.append(tuple(sorted(m + (d,))))
        prev_start, prev_len = out_start, len(mons) - out_start
    return mons, ops


MONS, GEN_OPS = _build_recipe()
NF = len(MONS)             # 164 for DEG=3


def _multinom(m):
    from math import factorial
    counts = {}
    for v in m:
        counts[v] = counts.get(v, 0) + 1
    r = factorial(len(m))
    for c in counts.values():
        r //= factorial(c)
    return r


MULTINOM = np.array([_multinom(m) for m in MONS], np.float64)
MON_DEG = np.array([len(m) for m in MONS], np.int64)


def _emit_gen_view(nc, F3, v, engine):
    """Emit the monomial-generation ops for view v (0=A at free offset 0,
    1=B at NF) into F3 [128, NSTRIP, 2*NF] on the given engine."""
    base = v * NF
    for (in_off, out_off, g, d) in GEN_OPS:
        engine.tensor_tensor(
            out=F3[:, :, base + out_off : base + out_off + g],
            in0=F3[:, :, base + in_off : base + in_off + g],
            in1=F3[:, :, base + d : base + d + 1].broadcast_to([128, NSTRIP, g]),
            op=mybir.AluOpType.mult,
        )


def _build_p1():
    f32 = mybir.dt.float32
    bf16 = mybir.dt.bfloat16
    nc = bacc.Bacc(None)
    coords = nc.dram_tensor("coords", [128, NSTRIP * 2 * D], bf16, kind="ExternalInput")
    out_psi = nc.dram_tensor("psi", [1, 2 * NF], f32, kind="ExternalOutput")

    with tile.TileContext(nc) as tc:
        with (
            tc.tile_pool(name="feat", bufs=1) as feat_pool,
            tc.tile_pool(name="small", bufs=1) as small_pool,
            tc.tile_pool(name="psum", bufs=1, space="PSUM") as psum_pool,
        ):
            F3 = feat_pool.tile([128, NSTRIP, 2 * NF], bf16)
            ones = small_pool.tile([128, 1], bf16)
            psi_sb = small_pool.tile([1, 2 * NF], f32)
            acc = psum_pool.tile([1, 2 * NF], f32)

            nc.vector.memset(ones[:], 1.0)
            # coords dram [p, (s, v, d)] -> degree-1 slots of F3
            nc.sync.dma_start(
                out=F3.rearrange("p s (v x) -> p s v x", v=2)[:, :, :, 0:D],
                in_=coords.rearrange("p (s v d) -> p s v d", v=2, d=D),
            )
            # view A monomials on DVE, view B on GPSIMD, concurrently; each
            # view's PE row-reduce starts as soon as that view's gen is done
            _emit_gen_view(nc, F3, 0, nc.vector)
            _emit_gen_view(nc, F3, 1, nc.gpsimd)
            for v in range(2):
                for s in range(NSTRIP):
                    nc.tensor.matmul(
                        acc[:, v * NF : (v + 1) * NF],
                        ones[:],
                        F3[:, s, v * NF : (v + 1) * NF],
                        start=(s == 0),
                        stop=(s == NSTRIP - 1),
                        skip_group_check=(v == 1),
                    )
                nc.scalar.copy(
                    out=psi_sb[:, v * NF : (v + 1) * NF],
                    in_=acc[:, v * NF : (v + 1) * NF],
                )
            nc.sync.dma_start(out=out_psi[:], in_=psi_sb[:])

    nc.compile()
    return nc


def _build_p2():
    f32 = mybir.dt.float32
    bf16 = mybir.dt.bfloat16
    nc = bacc.Bacc(None)
    coords = nc.dram_tensor("coords", [128, NSTRIP * 2 * D], bf16, kind="ExternalInput")
    wpair = nc.dram_tensor("wpair", [128, 2 * NF], bf16, kind="ExternalInput")
    out_sums = nc.dram_tensor("sums", [128, 2 * NSTRIP], f32, kind="ExternalOutput")

    with tile.TileContext(nc) as tc:
        with (
            tc.tile_pool(name="feat", bufs=1) as feat_pool,
            tc.tile_pool(name="small", bufs=1) as small_pool,
        ):
            F3 = feat_pool.tile([128, NSTRIP, 2 * NF], bf16)
            w_sb = small_pool.tile([128, 2 * NF], bf16)
            junk = small_pool.tile([128, NF], bf16)
            sums = small_pool.tile([128, 2 * NSTRIP], f32)

            junk2 = small_pool.tile([128, NF], bf16)

            nc.sync.dma_start(
                out=F3.rearrange("p s (v x) -> p s v x", v=2)[:, :, :, 0:D],
                in_=coords.rearrange("p (s v d) -> p s v d", v=2, d=D),
            )
            nc.sync.dma_start(out=w_sb[:], in_=wpair[:])
            # view A gen+dots on DVE, view B on GPSIMD; DVE (faster per op)
            # steals the last few view-B dots to balance the two engines
            _emit_gen_view(nc, F3, 0, nc.vector)
            _emit_gen_view(nc, F3, 1, nc.gpsimd)

            def dot(engine, jk, s, v):
                engine.scalar_tensor_tensor(
                    out=jk[:],
                    in0=F3[:, s, v * NF : (v + 1) * NF],
                    scalar=1.0,
                    in1=w_sb[:, v * NF : (v + 1) * NF],
                    op0=mybir.AluOpType.mult,
                    op1=mybir.AluOpType.mult,
                    accum_out=sums[:, 2 * s + v : 2 * s + v + 1],
                )

            for s in range(NSTRIP):
                dot(nc.vector, junk, s, 0)
            for s in range(NSTRIP):
                dot(nc.vector, junk, s, 1)
            nc.sync.dma_start(out=out_sums[:], in_=sums[:])

    nc.compile()
    return nc


_NC_CACHE = {}


def _get_nc(which):
    if which not in _NC_CACHE:
        _NC_CACHE[which] = _build_p1() if which == "p1" else _build_p2()
    return _NC_CACHE[which]


def _proj_np(z, W1, b1, W2, b2):
    h = z @ W1.T + b1
    h = np.where(h > 0, h, np.expm1(h)).astype(np.float32)
    return (h @ W2.T + b2).astype(np.float32)


def _prepare_operands(z_mp, z_sc, W1, b1, W2, b2):
    zp1 = _proj_np(z_mp.astype(np.float32), W1, b1, W2, b2)
    zp2 = _proj_np(z_sc.astype(np.float32), W1, b1, W2, b2)
    n1 = np.sqrt(np.sum(zp1 * zp1, axis=1, keepdims=True)).astype(np.float32)
    n2 = np.sqrt(np.sum(zp2 * zp2, axis=1, keepdims=True)).astype(np.float32)
    a = (zp1 / n1).astype(np.float32)
    b = (zp2 / (n2 * np.float32(TAU))).astype(np.float32)
    dots = np.sum(a.astype(np.float64) * b.astype(np.float64), axis=1)  # exact diag logits
    return a, b, dots


def _fit_poly(a, b):
    """Least-squares fit of exp on a subsample of the actual similarity
    distribution (the only consumer is log(sum), so ~1e-4 sum error is
    orders of magnitude inside the tolerance)."""
    xs = (a[::11].astype(np.float64) @ b[::13].astype(np.float64).T).ravel()
    V = np.vander(xs, DEG + 1, increasing=True)
    G = V.T @ V
    r = V.T @ np.exp(xs)
    return np.linalg.solve(G, r)  # c[0..DEG]


def _make_coords(a, b):
    """Pack per-core coords [128, (s, v, d)] in bf16, zero-padding the 30
    slots beyond the 1250 real rows (monomials of 0 are 0, so pads drop out
    of Psi/Phi automatically)."""
    out = []
    for k in range(NCORES):
        c = np.zeros((SLOTS, 2, D), np.float32)
        c[:RPC, 0, :] = a[k * RPC : (k + 1) * RPC]
        c[:RPC, 1, :] = b[k * RPC : (k + 1) * RPC]
        c = c.reshape(NSTRIP, 128, 2 * D).transpose(1, 0, 2).reshape(128, NSTRIP * 2 * D)
        out.append(np.ascontiguousarray(c.astype(BF16)))
    return out


def kernel(z_mp, z_sc, W1, b1, W2, b2):
    a, b, dots = _prepare_operands(z_mp, z_sc, W1, b1, W2, b2)
    c = _fit_poly(a, b)
    coords = _make_coords(a, b)

    nc1 = _get_nc("p1")
    res1 = run_bass_kernel_spmd(
        nc1, [{"coords": coords[k]} for k in range(NCORES)], list(range(NCORES))
    ).results
    partials = np.sum([np.asarray(res1[k]["psi"])[0] for k in range(NCORES)], axis=0)
    Phi = partials[:NF].astype(np.float64)   # sum_i phi(a_i)
    Psi = partials[NF:].astype(np.float64)   # sum_j phi(b_j)

    w = c[MON_DEG] * MULTINOM
    wpsi = (w * Psi).astype(np.float32)      # weights for the a-side dot (rowsum)
    wphi = (w * Phi).astype(np.float32)      # weights for the b-side dot (colsum)
    wpair = np.ascontiguousarray(
        np.tile(np.concatenate([wpsi, wphi]).astype(BF16)[None, :], (128, 1))
    )

    nc2 = _get_nc("p2")
    res2 = run_bass_kernel_spmd(
        nc2,
        [{"coords": coords[k], "wpair": wpair} for k in range(NCORES)],
        list(range(NCORES)),
    ).results

    row_sum = np.empty(N, np.float64)
    col_sum = np.empty(N, np.float64)
    for k in range(NCORES):
        s = np.asarray(res2[k]["sums"]).astype(np.float64)  # [128, 2*NSTRIP]
        rs = s[:, 0::2].T.reshape(-1)[:RPC]  # [p, s] -> row (s*128+p)
        cs = s[:, 1::2].T.reshape(-1)[:RPC]
        row_sum[k * RPC : (k + 1) * RPC] = rs
        col_sum[k * RPC : (k + 1) * RPC] = cs
    row_sum += c[0] * N + EPS
    col_sum += c[0] * N + EPS

    diag = np.exp(dots)
    lori_mp = -np.mean(np.log(diag / row_sum))
    lori_sc = -np.mean(np.log(diag / col_sum))
    return np.float32(LAM * lori_mp + (1.0 - LAM) * lori_sc)


# revision 9
# speedup vs baseline: 6.4710x; 1.6105x over previous
"""Trainium2 kernel for nn_Contrast: contrastive loss over the 10000x10000
exp-cosine-similarity matrix, via a polynomial kernel-feature expansion.

The loss only consumes the similarity matrix through per-row and per-column
sums of m = exp(a.b^T) (a = zp1/n1, b = zp2/(n2*tau)), plus the exact
diagonal.  exp is replaced by a least-squares polynomial p(x) = sum c_k x^k
fit on the empirical similarity distribution (deg 3 -> loss rel err ~4e-5,
deg 2 -> ~3e-4, tolerance 2e-2).  With phi the vector of monomials of degree
1..DEG in the 8 coordinates,

    rowsum_i ~= c0*N + sum_alpha w_alpha phi_alpha(a_i) * Psi_alpha,
    Psi_alpha = sum_j phi_alpha(b_j),   w_alpha = c_|alpha| * multinomial(alpha)

and symmetrically colsum_j with Phi = sum_i phi_alpha(a_i).  This is O(N*NF)
instead of O(N^2): no N x N matrix and no 1e8 exp() evaluations.

Device structure (rows sharded 1250/core across 8 cores, two launches):
  P1: monomial generation for the core's a- and b-shard (strip-batched
      tensor_tensor with a broadcast coordinate operand, split DVE/GPSIMD),
      then PE reduces over rows (ones-matmul, PSUM-accumulated over strip
      groups) -> per-core partials [Phi | Psi].  Host sums 8 tiny partials.
  P2: regenerates the monomials, multiplies by the broadcast w*Psi / w*Phi
      vectors (tensor_tensor, 2x DVE mode on packed bf16), and tensor_reduce
      over the feature axis emits the per-row / per-column sums.
Host does only O(N*D) prep (projection, norms, exact diagonal — same as the
exact-kernel baseline), the tiny poly fit, and the O(N) log/mean finalize.
"""

import numpy as np
import ml_dtypes

import concourse.bass as bass
import concourse.bacc as bacc
import concourse.mybir as mybir
import concourse.tile as tile
from concourse.bass_utils import run_bass_kernel_spmd

TAU = 0.5
LAM = 0.5
EPS = 1e-8

N = 10000
D = 8
NCORES = 8
RPC = N // NCORES          # 1250 real rows per core
NSTRIP = 10                # 10 strips x 128 partitions = 1280 slots (30 pad)
SLOTS = NSTRIP * 128
DEG = 2

BF16 = ml_dtypes.bfloat16


def _build_recipe():
    """Monomial ordering: degree-major; within a degree, grouped by max
    variable index so each degree-k/maxvar-d block is (prefix of the
    degree-(k-1) block) * x_d.  Returns (mons, ops) where ops entries are
    (k, in_off, out_off, g, d) with offsets into the full monomial list."""
    mons = [(d,) for d in range(D)]
    ops = []
    prev_start, prev_len = 0, D
    for k in range(2, DEG + 1):
        out_start = len(mons)
        for d in range(D):
            g = sum(1 for m in mons[prev_start:prev_start + prev_len] if max(m) <= d)
            if g == 0:
                continue
            ops.append((k, prev_start, len(mons), g, d))
            for m in mons[prev_start:prev_start + g]:
                mons.append(tuple(sorted(m + (d,))))
        prev_start, prev_len = out_start, len(mons) - out_start
    return mons, ops


MONS, GEN_OPS = _build_recipe()
NF = len(MONS)             # 44 for DEG=2, 164 for DEG=3
SGRP = min(NSTRIP, 512 // NF)   # strips per PSUM-bank matmul group
MM_GROUPS = [(s0, min(SGRP, NSTRIP - s0)) for s0 in range(0, NSTRIP, SGRP)]


def _multinom(m):
    from math import factorial
    counts = {}
    for v in m:
        counts[v] = counts.get(v, 0) + 1
    r = factorial(len(m))
    for c in counts.values():
        r //= factorial(c)
    return r


MULTINOM = np.array([_multinom(m) for m in MONS], np.float64)
MON_DEG = np.array([len(m) for m in MONS], np.int64)

# gen split: GPSIMD (Pool) is ~1.9x slower per element, so it gets view B
# minus the largest top-degree blocks, which go to DVE after view A
POOL_OPS = [op for op in GEN_OPS if not (op[0] == DEG and op[4] >= 7)]
DVE_B_OPS = [op for op in GEN_OPS if (op[0] == DEG and op[4] >= 7)]


def _emit_gen(nc, F3, v, engine, ops):
    base = v * NF
    for (_k, in_off, out_off, g, d) in ops:
        engine.tensor_tensor(
            out=F3[:, :, base + out_off : base + out_off + g],
            in0=F3[:, :, base + in_off : base + in_off + g],
            in1=F3[:, :, base + d : base + d + 1].broadcast_to([128, NSTRIP, g]),
            op=mybir.AluOpType.mult,
        )


def _emit_coords_load(nc, coords, stage, F3):
    """Contiguous DMA into a staging tile (1 descriptor/partition), then a
    cheap on-chip copy scatters the degree-1 slots into F3."""
    nc.sync.dma_start(out=stage[:], in_=coords[:])
    nc.vector.tensor_copy(
        out=F3.rearrange("p s (v x) -> p s v x", v=2)[:, :, :, 0:D],
        in_=stage.rearrange("p (s v d) -> p s v d", v=2, d=D),
    )


def _build_p1():
    f32 = mybir.dt.float32
    bf16 = mybir.dt.bfloat16
    nc = bacc.Bacc(None)
    coords = nc.dram_tensor("coords", [128, NSTRIP * 2 * D], bf16, kind="ExternalInput")
    out_psi = nc.dram_tensor("psi", [1, 2 * SGRP * NF], f32, kind="ExternalOutput")

    with tile.TileContext(nc) as tc:
        with (
            tc.tile_pool(name="feat", bufs=1) as feat_pool,
            tc.tile_pool(name="small", bufs=1) as small_pool,
            tc.tile_pool(name="psum", bufs=1, space="PSUM") as psum_pool,
        ):
            F3 = feat_pool.tile([128, NSTRIP, 2 * NF], bf16)
            stage = small_pool.tile([128, NSTRIP * 2 * D], bf16)
            ones = small_pool.tile([128, 1], bf16)
            psi_sb = small_pool.tile([1, 2 * SGRP * NF], f32)
            acc = [psum_pool.tile([1, SGRP, NF], f32, name=f"acc{v}") for v in range(2)]

            nc.vector.memset(ones[:], 1.0)
            _emit_coords_load(nc, coords, stage, F3)
            _emit_gen(nc, F3, 0, nc.vector, GEN_OPS)
            _emit_gen(nc, F3, 1, nc.gpsimd, POOL_OPS)
            _emit_gen(nc, F3, 1, nc.vector, DVE_B_OPS)
            for v in range(2):
                for gi, (s0, ns) in enumerate(MM_GROUPS):
                    nc.tensor.matmul(
                        acc[v][:, 0:ns, :],
                        ones[:],
                        F3[:, s0 : s0 + ns, v * NF : (v + 1) * NF],
                        start=(gi == 0),
                        stop=(gi == len(MM_GROUPS) - 1),
                        skip_group_check=True,
                    )
                hw = SGRP * NF
                nc.scalar.copy(
                    out=psi_sb[:, v * hw : (v + 1) * hw],
                    in_=acc[v].rearrange("o s f -> o (s f)"),
                )
                nc.sync.dma_start(
                    out=out_psi[:, v * hw : (v + 1) * hw],
                    in_=psi_sb[:, v * hw : (v + 1) * hw],
                )

    nc.compile()
    return nc


def _build_p2():
    f32 = mybir.dt.float32
    bf16 = mybir.dt.bfloat16
    nc = bacc.Bacc(None)
    coords = nc.dram_tensor("coords", [128, NSTRIP * 2 * D], bf16, kind="ExternalInput")
    wpair = nc.dram_tensor("wpair", [128, 2 * NF], bf16, kind="ExternalInput")
    out_rs = nc.dram_tensor("rowsums", [128, NSTRIP], f32, kind="ExternalOutput")
    out_cs = nc.dram_tensor("colsums", [128, NSTRIP], f32, kind="ExternalOutput")

    with tile.TileContext(nc) as tc:
        with (
            tc.tile_pool(name="feat", bufs=1) as feat_pool,
            tc.tile_pool(name="small", bufs=1) as small_pool,
        ):
            F3 = feat_pool.tile([128, NSTRIP, 2 * NF], bf16)
            prod = feat_pool.tile([128, NSTRIP, 2 * NF], bf16)
            stage = small_pool.tile([128, NSTRIP * 2 * D], bf16)
            w_sb = small_pool.tile([128, 2 * NF], bf16)
            sums = [small_pool.tile([128, NSTRIP], f32, name=f"sums{v}") for v in range(2)]

            _emit_coords_load(nc, coords, stage, F3)
            nc.sync.dma_start(out=w_sb[:], in_=wpair[:])
            _emit_gen(nc, F3, 0, nc.vector, GEN_OPS)
            _emit_gen(nc, F3, 1, nc.gpsimd, GEN_OPS)
            outs = [out_rs, out_cs]
            for v in range(2):
                lo, hi = v * NF, (v + 1) * NF
                nc.vector.tensor_tensor(
                    out=prod[:, :, lo:hi],
                    in0=F3[:, :, lo:hi],
                    in1=w_sb[:, None, lo:hi].broadcast_to([128, NSTRIP, NF]),
                    op=mybir.AluOpType.mult,
                )
            for v in range(2):
                lo, hi = v * NF, (v + 1) * NF
                nc.vector.tensor_reduce(
                    out=sums[v][:, :],
                    in_=prod[:, :, lo:hi],
                    axis=mybir.AxisListType.X,
                    op=mybir.AluOpType.add,
                )
                nc.sync.dma_start(out=outs[v][:], in_=sums[v][:])

    nc.compile()
    return nc


_NC_CACHE = {}


def _get_nc(which):
    if which not in _NC_CACHE:
        _NC_CACHE[which] = _build_p1() if which == "p1" else _build_p2()
    return _NC_CACHE[which]


def _proj_np(z, W1, b1, W2, b2):
    h = z @ W1.T + b1
    h = np.where(h > 0, h, np.expm1(h)).astype(np.float32)
    return (h @ W2.T + b2).astype(np.float32)


def _prepare_operands(z_mp, z_sc, W1, b1, W2, b2):
    zp1 = _proj_np(z_mp.astype(np.float32), W1, b1, W2, b2)
    zp2 = _proj_np(z_sc.astype(np.float32), W1, b1, W2, b2)
    n1 = np.sqrt(np.sum(zp1 * zp1, axis=1, keepdims=True)).astype(np.float32)
    n2 = np.sqrt(np.sum(zp2 * zp2, axis=1, keepdims=True)).astype(np.float32)
    a = (zp1 / n1).astype(np.float32)
    b = (zp2 / (n2 * np.float32(TAU))).astype(np.float32)
    dots = np.sum(a.astype(np.float64) * b.astype(np.float64), axis=1)  # exact diag logits
    return a, b, dots


def _fit_poly(a, b):
    """Least-squares fit of exp on a subsample of the actual similarity
    distribution (the only consumer is log(sum), so ~1e-4 sum error is
    orders of magnitude inside the tolerance)."""
    xs = (a[::11].astype(np.float64) @ b[::13].astype(np.float64).T).ravel()
    V = np.vander(xs, DEG + 1, increasing=True)
    G = V.T @ V
    r = V.T @ np.exp(xs)
    return np.linalg.solve(G, r)  # c[0..DEG]


def _make_coords(a, b):
    """Pack per-core coords [128, (s, v, d)] in bf16, zero-padding the 30
    slots beyond the 1250 real rows (monomials of 0 are 0, so pads drop out
    of Psi/Phi automatically)."""
    out = []
    for k in range(NCORES):
        c = np.zeros((SLOTS, 2, D), np.float32)
        c[:RPC, 0, :] = a[k * RPC : (k + 1) * RPC]
        c[:RPC, 1, :] = b[k * RPC : (k + 1) * RPC]
        c = c.reshape(NSTRIP, 128, 2 * D).transpose(1, 0, 2).reshape(128, NSTRIP * 2 * D)
        out.append(np.ascontiguousarray(c.astype(BF16)))
    return out


def kernel(z_mp, z_sc, W1, b1, W2, b2):
    a, b, dots = _prepare_operands(z_mp, z_sc, W1, b1, W2, b2)
    c = _fit_poly(a, b)
    coords = _make_coords(a, b)

    nc1 = _get_nc("p1")
    res1 = run_bass_kernel_spmd(
        nc1, [{"coords": coords[k]} for k in range(NCORES)], list(range(NCORES))
    ).results
    # psi[v] is [2, SGRP*NF]; sum cores and the SGRP strip-group slices
    partials = np.sum(
        [np.asarray(res1[k]["psi"]).astype(np.float64) for k in range(NCORES)], axis=0
    ).reshape(2, SGRP, NF).sum(axis=1)
    Phi, Psi = partials[0], partials[1]   # sum_i phi(a_i), sum_j phi(b_j)

    w = c[MON_DEG] * MULTINOM
    wpsi = (w * Psi).astype(np.float32)      # weights for the a-side dot (rowsum)
    wphi = (w * Phi).astype(np.float32)      # weights for the b-side dot (colsum)
    wpair = np.ascontiguousarray(
        np.tile(np.concatenate([wpsi, wphi]).astype(BF16)[None, :], (128, 1))
    )

    nc2 = _get_nc("p2")
    res2 = run_bass_kernel_spmd(
        nc2,
        [{"coords": coords[k], "wpair": wpair} for k in range(NCORES)],
        list(range(NCORES)),
    ).results

    row_sum = np.empty(N, np.float64)
    col_sum = np.empty(N, np.float64)
    for k in range(NCORES):
        rs = np.asarray(res2[k]["rowsums"]).astype(np.float64)  # [128, NSTRIP]
        cs = np.asarray(res2[k]["colsums"]).astype(np.float64)
        row_sum[k * RPC : (k + 1) * RPC] = rs.T.reshape(-1)[:RPC]  # row = s*128+p
        col_sum[k * RPC : (k + 1) * RPC] = cs.T.reshape(-1)[:RPC]
    row_sum += c[0] * N + EPS
    col_sum += c[0] * N + EPS

    diag = np.exp(dots)
    lori_mp = -np.mean(np.log(diag / row_sum))
    lori_sc = -np.mean(np.log(diag / col_sum))
    return np.float32(LAM * lori_mp + (1.0 - LAM) * lori_sc)


# revision 12
# speedup vs baseline: 6.8122x; 1.0527x over previous
"""Trainium2 kernel for nn_Contrast: contrastive loss over the 10000x10000
exp-cosine-similarity matrix, via a polynomial kernel-feature expansion.

The loss only consumes the similarity matrix through per-row and per-column
sums of m = exp(a.b^T) (a = zp1/n1, b = zp2/(n2*tau)), plus the exact
diagonal.  exp is replaced by a least-squares polynomial p(x) = sum c_k x^k
fit on the empirical similarity distribution (deg 3 -> loss rel err ~4e-5,
deg 2 -> ~3e-4, tolerance 2e-2).  With phi the vector of monomials of degree
1..DEG in the 8 coordinates,

    rowsum_i ~= c0*N + sum_alpha w_alpha phi_alpha(a_i) * Psi_alpha,
    Psi_alpha = sum_j phi_alpha(b_j),   w_alpha = c_|alpha| * multinomial(alpha)

and symmetrically colsum_j with Phi = sum_i phi_alpha(a_i).  This is O(N*NF)
instead of O(N^2): no N x N matrix and no 1e8 exp() evaluations.

Device structure (rows sharded 1250/core across 8 cores, two launches):
  P1: monomial generation for the core's a- and b-shard (strip-batched
      tensor_tensor with a broadcast coordinate operand, split DVE/GPSIMD),
      then PE reduces over rows (ones-matmul, PSUM-accumulated over strip
      groups) -> per-core partials [Phi | Psi].  Host sums 8 tiny partials.
  P2: regenerates the monomials, multiplies by the broadcast w*Psi / w*Phi
      vectors (tensor_tensor, 2x DVE mode on packed bf16), and tensor_reduce
      over the feature axis emits the per-row / per-column sums.
Host does only O(N*D) prep (projection, norms, exact diagonal — same as the
exact-kernel baseline), the tiny poly fit, and the O(N) log/mean finalize.
"""

import numpy as np
import ml_dtypes

import concourse.bass as bass
import concourse.bacc as bacc
import concourse.mybir as mybir
import concourse.tile as tile
from concourse.bass_utils import run_bass_kernel_spmd

TAU = 0.5
LAM = 0.5
EPS = 1e-8

N = 10000
D = 8
NCORES = 8
RPC = N // NCORES          # 1250 real rows per core
NSTRIP = 10                # 10 strips x 128 partitions = 1280 slots (30 pad)
SLOTS = NSTRIP * 128
DEG = 2

BF16 = ml_dtypes.bfloat16


def _build_recipe():
    """Monomial ordering: degree-major; within a degree, grouped by max
    variable index so each degree-k/maxvar-d block is (prefix of the
    degree-(k-1) block) * x_d.  Returns (mons, ops) where ops entries are
    (k, in_off, out_off, g, d) with offsets into the full monomial list."""
    mons = [(d,) for d in range(D)]
    ops = []
    prev_start, prev_len = 0, D
    for k in range(2, DEG + 1):
        out_start = len(mons)
        for d in range(D):
            g = sum(1 for m in mons[prev_start:prev_start + prev_len] if max(m) <= d)
            if g == 0:
                continue
            ops.append((k, prev_start, len(mons), g, d))
            for m in mons[prev_start:prev_start + g]:
                mons.append(tuple(sorted(m + (d,))))
        prev_start, prev_len = out_start, len(mons) - out_start
    return mons, ops


MONS, GEN_OPS = _build_recipe()
NF = len(MONS)             # 44 for DEG=2, 164 for DEG=3
SGRP = min(NSTRIP, 512 // NF)   # strips per PSUM-bank matmul group
MM_GROUPS = [(s0, min(SGRP, NSTRIP - s0)) for s0 in range(0, NSTRIP, SGRP)]


def _multinom(m):
    from math import factorial
    counts = {}
    for v in m:
        counts[v] = counts.get(v, 0) + 1
    r = factorial(len(m))
    for c in counts.values():
        r //= factorial(c)
    return r


MULTINOM = np.array([_multinom(m) for m in MONS], np.float64)
MON_DEG = np.array([len(m) for m in MONS], np.int64)

# gen split: GPSIMD (Pool) is ~1.9x slower per element, so it gets view B
# minus the largest top-degree blocks, which go to DVE after view A
POOL_OPS = [op for op in GEN_OPS if not (op[0] == DEG and op[4] >= 7)]
DVE_B_OPS = [op for op in GEN_OPS if (op[0] == DEG and op[4] >= 7)]


def _emit_gen(nc, F3, v, engine, ops):
    base = v * NF
    for (_k, in_off, out_off, g, d) in ops:
        engine.tensor_tensor(
            out=F3[:, :, base + out_off : base + out_off + g],
            in0=F3[:, :, base + in_off : base + in_off + g],
            in1=F3[:, :, base + d : base + d + 1].broadcast_to([128, NSTRIP, g]),
            op=mybir.AluOpType.mult,
        )


def _emit_coords_load(nc, coords, stage, F3):
    """Contiguous DMA into a staging tile (1 descriptor/partition), then a
    cheap on-chip copy scatters the degree-1 slots into F3."""
    nc.sync.dma_start(out=stage[:], in_=coords[:])
    nc.vector.tensor_copy(
        out=F3.rearrange("p s (v x) -> p s v x", v=2)[:, :, :, 0:D],
        in_=stage.rearrange("p (s v d) -> p s v d", v=2, d=D),
    )


def _build_p1():
    f32 = mybir.dt.float32
    bf16 = mybir.dt.bfloat16
    nc = bacc.Bacc(None)
    coords = nc.dram_tensor("coords", [128, NSTRIP * 2 * D], bf16, kind="ExternalInput")
    out_psi = nc.dram_tensor("psi", [1, 2 * SGRP * NF], f32, kind="ExternalOutput")

    with tile.TileContext(nc) as tc:
        with (
            tc.tile_pool(name="feat", bufs=1) as feat_pool,
            tc.tile_pool(name="small", bufs=1) as small_pool,
            tc.tile_pool(name="psum", bufs=1, space="PSUM") as psum_pool,
        ):
            F3 = feat_pool.tile([128, NSTRIP, 2 * NF], bf16)
            stage = small_pool.tile([128, NSTRIP * 2 * D], bf16)
            ones = small_pool.tile([128, 1], bf16)
            psi_sb = small_pool.tile([1, 2 * SGRP * NF], f32)
            acc = [psum_pool.tile([1, SGRP, NF], f32, name=f"acc{v}") for v in range(2)]

            nc.vector.memset(ones[:], 1.0)
            _emit_coords_load(nc, coords, stage, F3)
            _emit_gen(nc, F3, 0, nc.vector, GEN_OPS)
            _emit_gen(nc, F3, 1, nc.gpsimd, POOL_OPS)
            _emit_gen(nc, F3, 1, nc.vector, DVE_B_OPS)
            hw = SGRP * NF
            for v in range(2):
                for gi, (s0, ns) in enumerate(MM_GROUPS):
                    nc.tensor.matmul(
                        acc[v][:, 0:ns, :],
                        ones[:],
                        F3[:, s0 : s0 + ns, v * NF : (v + 1) * NF],
                        start=(gi == 0),
                        stop=(gi == len(MM_GROUPS) - 1),
                        skip_group_check=True,
                    )
            # PSUM -> SBUF on two different engines so the copies overlap,
            # then a single output DMA
            nc.scalar.copy(
                out=psi_sb[:, 0:hw], in_=acc[0].rearrange("o s f -> o (s f)")
            )
            nc.vector.tensor_copy(
                out=psi_sb[:, hw : 2 * hw], in_=acc[1].rearrange("o s f -> o (s f)")
            )
            nc.sync.dma_start(out=out_psi[:], in_=psi_sb[:])

    nc.compile()
    return nc


def _build_p2():
    f32 = mybir.dt.float32
    bf16 = mybir.dt.bfloat16
    nc = bacc.Bacc(None)
    coords = nc.dram_tensor("coords", [128, NSTRIP * 2 * D], bf16, kind="ExternalInput")
    wpair = nc.dram_tensor("wpair", [128, 2 * NF], bf16, kind="ExternalInput")
    out_sums = nc.dram_tensor("sums", [128, 2 * NSTRIP], f32, kind="ExternalOutput")

    with tile.TileContext(nc) as tc:
        with (
            tc.tile_pool(name="feat", bufs=1) as feat_pool,
            tc.tile_pool(name="small", bufs=1) as small_pool,
        ):
            F3 = feat_pool.tile([128, NSTRIP, 2 * NF], bf16)
            prod = feat_pool.tile([128, NSTRIP, 2 * NF], bf16)
            stage = small_pool.tile([128, NSTRIP * 2 * D], bf16)
            w_sb = small_pool.tile([128, 2 * NF], bf16)
            sums = small_pool.tile([128, 2 * NSTRIP], f32)

            _emit_coords_load(nc, coords, stage, F3)
            nc.sync.dma_start(out=w_sb[:], in_=wpair[:])
            _emit_gen(nc, F3, 0, nc.vector, GEN_OPS)
            _emit_gen(nc, F3, 1, nc.gpsimd, POOL_OPS)
            _emit_gen(nc, F3, 1, nc.vector, DVE_B_OPS)
            # pipelined per-view: the A product/reduce only depend on the DVE
            # gen, so they overlap the tail of the GPSIMD gen of view B
            for v in range(2):
                lo, hi = v * NF, (v + 1) * NF
                nc.vector.tensor_tensor(
                    out=prod[:, :, lo:hi],
                    in0=F3[:, :, lo:hi],
                    in1=w_sb[:, None, lo:hi].broadcast_to([128, NSTRIP, NF]),
                    op=mybir.AluOpType.mult,
                )
                nc.vector.tensor_reduce(
                    out=sums[:, v * NSTRIP : (v + 1) * NSTRIP],
                    in_=prod[:, :, lo:hi],
                    axis=mybir.AxisListType.X,
                    op=mybir.AluOpType.add,
                )
            nc.sync.dma_start(out=out_sums[:], in_=sums[:])

    nc.compile()
    return nc


_NC_CACHE = {}


def _get_nc(which):
    if which not in _NC_CACHE:
        _NC_CACHE[which] = _build_p1() if which == "p1" else _build_p2()
    return _NC_CACHE[which]


def _proj_np(z, W1, b1, W2, b2):
    h = z @ W1.T + b1
    h = np.where(h > 0, h, np.expm1(h)).astype(np.float32)
    return (h @ W2.T + b2).astype(np.float32)


def _prepare_operands(z_mp, z_sc, W1, b1, W2, b2):
    zp1 = _proj_np(z_mp.astype(np.float32), W1, b1, W2, b2)
    zp2 = _proj_np(z_sc.astype(np.float32), W1, b1, W2, b2)
    n1 = np.sqrt(np.sum(zp1 * zp1, axis=1, keepdims=True)).astype(np.float32)
    n2 = np.sqrt(np.sum(zp2 * zp2, axis=1, keepdims=True)).astype(np.float32)
    a = (zp1 / n1).astype(np.float32)
    b = (zp2 / (n2 * np.float32(TAU))).astype(np.float32)
    dots = np.sum(a.astype(np.float64) * b.astype(np.float64), axis=1)  # exact diag logits
    return a, b, dots


def _fit_poly(a, b):
    """Least-squares fit of exp on a subsample of the actual similarity
    distribution (the only consumer is log(sum), so ~1e-4 sum error is
    orders of magnitude inside the tolerance)."""
    xs = (a[::11].astype(np.float64) @ b[::13].astype(np.float64).T).ravel()
    V = np.vander(xs, DEG + 1, increasing=True)
    G = V.T @ V
    r = V.T @ np.exp(xs)
    return np.linalg.solve(G, r)  # c[0..DEG]


def _make_coords(a, b):
    """Pack per-core coords [128, (s, v, d)] in bf16, zero-padding the 30
    slots beyond the 1250 real rows (monomials of 0 are 0, so pads drop out
    of Psi/Phi automatically)."""
    out = []
    for k in range(NCORES):
        c = np.zeros((SLOTS, 2, D), np.float32)
        c[:RPC, 0, :] = a[k * RPC : (k + 1) * RPC]
        c[:RPC, 1, :] = b[k * RPC : (k + 1) * RPC]
        c = c.reshape(NSTRIP, 128, 2 * D).transpose(1, 0, 2).reshape(128, NSTRIP * 2 * D)
        out.append(np.ascontiguousarray(c.astype(BF16)))
    return out


def kernel(z_mp, z_sc, W1, b1, W2, b2):
    a, b, dots = _prepare_operands(z_mp, z_sc, W1, b1, W2, b2)
    c = _fit_poly(a, b)
    coords = _make_coords(a, b)

    nc1 = _get_nc("p1")
    res1 = run_bass_kernel_spmd(
        nc1, [{"coords": coords[k]} for k in range(NCORES)], list(range(NCORES))
    ).results
    # psi[v] is [2, SGRP*NF]; sum cores and the SGRP strip-group slices
    partials = np.sum(
        [np.asarray(res1[k]["psi"]).astype(np.float64) for k in range(NCORES)], axis=0
    ).reshape(2, SGRP, NF).sum(axis=1)
    Phi, Psi = partials[0], partials[1]   # sum_i phi(a_i), sum_j phi(b_j)

    w = c[MON_DEG] * MULTINOM
    wpsi = (w * Psi).astype(np.float32)      # weights for the a-side dot (rowsum)
    wphi = (w * Phi).astype(np.float32)      # weights for the b-side dot (colsum)
    wpair = np.ascontiguousarray(
        np.tile(np.concatenate([wpsi, wphi]).astype(BF16)[None, :], (128, 1))
    )

    nc2 = _get_nc("p2")
    res2 = run_bass_kernel_spmd(
        nc2,
        [{"coords": coords[k], "wpair": wpair} for k in range(NCORES)],
        list(range(NCORES)),
    ).results

    row_sum = np.empty(N, np.float64)
    col_sum = np.empty(N, np.float64)
    for k in range(NCORES):
        s = np.asarray(res2[k]["sums"]).astype(np.float64)  # [128, 2*NSTRIP]
        row_sum[k * RPC : (k + 1) * RPC] = s[:, :NSTRIP].T.reshape(-1)[:RPC]
        col_sum[k * RPC : (k + 1) * RPC] = s[:, NSTRIP:].T.reshape(-1)[:RPC]
    row_sum += c[0] * N + EPS
    col_sum += c[0] * N + EPS

    diag = np.exp(dots)
    lori_mp = -np.mean(np.log(diag / row_sum))
    lori_sc = -np.mean(np.log(diag / col_sum))
    return np.float32(LAM * lori_mp + (1.0 - LAM) * lori_sc)
